# revision 9
# baseline (speedup 1.0000x reference)
"""CBOW model (embedding gather -> mean -> logits -> softmax) on 8 Trainium2
NeuronCores.

Sharding strategy (model/vocab parallel, per the hint):
  - W1 and W2 are both sharded along the vocab axis: core m owns W1 rows
    [m*12500, (m+1)*12500) (fp16, plus an appended zero row) and W2 columns
    [m*12500, (m+1)*12500) (fp16, padded to 12544 with zero columns).  Inputs
    shipped per core are ~6.5 MB instead of a replicated ~57.6 MB.
  - Gather: every core looks up ALL 2048x10 indices against its own W1 shard;
    out-of-shard indices are remapped (host-side) to the zero row, so the
    per-core context sums are partial sums.  The context sum happens on the
    PE via accumulating transposes; chunked AllReduces(add) over the
    transposed partial hidden give every core the full [128, 2048] hidden.
    The emission is software-pipelined (prologue of chunk k+2 is emitted
    before pass 1 of chunk k) so only the first AllReduce is exposed;
    collectives issue from the scalar queue so the gather stream on the
    gpsimd queue is never head-blocked.
  - Softmax: pass 1 computes the logit shard (written out as fp16) and
    per-row exp-sums on the f32 PSUM logits (exact Z; fused into the Exp
    activation via accum_out); AllReduces(add) of the per-row local sums
    (in two halves, so pass 2 can start while pass 1 finishes) give the
    global denominator; pass 2 recomputes the matmul tiles and applies
    exp(logit - log(sum)) via the Exp activation's per-partition bias,
    writing the softmax block as bf16.  Max-subtraction is not needed:
    |logit| < 40 always, exp() is safe in f32.  Matmuls run in fp16
    (1 cycle/row on the PE; fp32 needs 4).
"""

import numpy as np

import concourse.bass as bass
import concourse.mybir as mybir
import concourse.tile as tile
from concourse import bacc
from concourse.masks import make_identity
import concourse.bass_utils as bass_utils

# Problem shape (hardcoded; matches reference.setup_inputs()).
V = 100000      # vocab
D = 128         # embed dim
B = 2048        # batch
C = 10          # context positions
M = 8           # cores
S = V // M      # vocab shard per core = 12500
SP = 12544      # shard padded so every matmul chunk is >= 256 wide
P = 128         # partitions
BT = B // P     # batch tiles = 16
MMN = 512       # max moving free dim per matmul (one PSUM bank, f32)
GRP = 2048      # vocab columns per PSUM group (4 banks)

F32 = mybir.dt.float32
F16 = mybir.dt.float16
BF16 = mybir.dt.bfloat16
I32 = mybir.dt.int32
AF = mybir.ActivationFunctionType

# (start, width) vocab-column groups per core; width <= GRP.
GROUPS = [(g0, min(GRP, SP - g0)) for g0 in range(0, SP, GRP)]
# groups whose PSUM->SBUF logits cast runs on the scalar engine instead of
# DVE, to balance the two engines in pass 1 (ACT also does every group's exp)
ACT_COPY_GROUPS = {len(GROUPS) - 1, len(GROUPS) - 2}
# batch tiles per hidden-AllReduce chunk: a 1-tile first chunk minimizes the
# exposed pipeline head; later gathers/AllReduces overlap pass-1 compute
CHUNKS = [1, 3, 4, 4, 4]
NCH = len(CHUNKS)
CH_START = [sum(CHUNKS[:k]) for k in range(NCH)]


def build_nc(n_cores: int = M):
    nc = bacc.Bacc("TRN2", target_bir_lowering=False, debug=False,
                   num_devices=n_cores)

    w1s = nc.dram_tensor("w1s", [S + 1, D], F16, kind="ExternalInput")
    w2s = nc.dram_tensor("w2s", [P, SP], F16, kind="ExternalInput")
    idxs = nc.dram_tensor("idxs", [P, BT * C], I32, kind="ExternalInput")
    logits_s = nc.dram_tensor("logits_s", [B, S], F16, kind="ExternalOutput")
    soft_s = nc.dram_tensor("soft_s", [B, S], BF16, kind="ExternalOutput")

    rg = [list(range(n_cores))]

    with tile.TileContext(nc) as tc:
        with tc.tile_pool(name="sbuf", bufs=1) as sbuf, \
             tc.tile_pool(name="gathp", bufs=4) as gathp, \
             tc.tile_pool(name="hidp", bufs=2) as hidp, \
             tc.tile_pool(name="stagp", bufs=2) as stagp, \
             tc.tile_pool(name="psum", bufs=2, space="PSUM") as psum, \
             tc.tile_pool(name="dram", bufs=1, space="DRAM") as dram:
            idx_sb = sbuf.tile([P, BT * C], I32)
            nc.sync.dma_start(out=idx_sb[:], in_=idxs[:])

            ident = sbuf.tile([P, P], F16)
            make_identity(nc, ident[:])

            # W2 shard resident in SBUF for both passes.
            w2_sb = sbuf.tile([P, SP], F16)
            nc.sync.dma_start(out=w2_sb[:], in_=w2s[:])

            hidT = []        # per-chunk [D, ct*128] full transposed hidden
            lsum = sbuf.tile([P, BT], F32)

            def prologue_chunk(k):
                ct = CHUNKS[k]
                t0 = CH_START[k]
                hch = hidp.tile([P, ct * P], F16, tag=f"hch{ct}")
                for tt in range(ct):
                    t = t0 + tt
                    gath = gathp.tile([P, C * D], F16, tag="gath")
                    for c in range(C):
                        j = t * C + c
                        nc.gpsimd.indirect_dma_start(
                            out=gath[:, c * D:(c + 1) * D],
                            out_offset=None,
                            in_=w1s[:],
                            in_offset=bass.IndirectOffsetOnAxis(
                                ap=idx_sb[:, j:j + 1], axis=0),
                        )
                    hid = hidp.tile([P, D], F32, tag="hid")
                    nc.vector.tensor_reduce(
                        out=hid[:],
                        in_=gath[:].rearrange("p (c d) -> p d c", c=C),
                        axis=mybir.AxisListType.X,
                        op=mybir.AluOpType.add,
                    )
                    hid16 = hidp.tile([P, D], F16, tag="hid16")
                    # context mean folded in here (x 1/10)
                    nc.vector.tensor_scalar_mul(hid16[:], hid[:], 1.0 / C)
                    tp = psum.tile([P, 2 * GRP], F16, tag="mm")
                    nc.tensor.transpose(out=tp[:, :P], in_=hid16[:],
                                        identity=ident[:])
                    nc.vector.tensor_copy(hch[:, tt * P:(tt + 1) * P],
                                          tp[:, :P])
                cc_in = dram.tile([P, ct * P], F16)
                cc_out = dram.tile(
                    [P, ct * P], F16,
                    addr_space="Shared" if n_cores > 1 else "Local")
                nc.sync.dma_start(out=cc_in[:], in_=hch[:])
                if n_cores > 1:
                    nc.gpsimd.collective_compute(
                        "AllReduce", mybir.AluOpType.add, replica_groups=rg,
                        ins=[cc_in[:]], outs=[cc_out[:]],
                    )
                else:
                    nc.gpsimd.dma_start(out=cc_out[:], in_=cc_in[:])
                ht = sbuf.tile([P, ct * P], F16, name=f"hidT{k}")
                nc.sync.dma_start(out=ht[:], in_=cc_out[:])
                hidT.append(ht)

            def pass1_chunk(k):
                ct = CHUNKS[k]
                t0 = CH_START[k]
                for tt in range(ct):
                    t = t0 + tt
                    lhsT = hidT[k][:, tt * P:(tt + 1) * P]
                    stag = stagp.tile([P, SP], F16, tag="stag1")
                    sums = hidp.tile([P, len(GROUPS)], F32, tag="sums")
                    for gi, (g0, gw) in enumerate(GROUPS):
                        ps = psum.tile([P, GRP], F32, tag="mm")
                        for s0 in range(0, gw, MMN):
                            w = min(MMN, gw - s0)
                            nc.tensor.matmul(
                                out=ps[:, s0:s0 + w], lhsT=lhsT,
                                rhs=w2_sb[:, g0 + s0:g0 + s0 + w],
                                start=True, stop=True)
                        escr = stagp.tile([P, GRP], BF16, tag="escr")
                        nc.scalar.activation(
                            out=escr[:, :gw], in_=ps[:, :gw], func=AF.Exp,
                            accum_out=sums[:, gi:gi + 1])
                        if gi in ACT_COPY_GROUPS:
                            nc.scalar.copy(stag[:, g0:g0 + gw], ps[:, :gw])
                        else:
                            nc.vector.tensor_copy(stag[:, g0:g0 + gw],
                                                  ps[:, :gw])
                    nc.vector.tensor_reduce(
                        out=lsum[:, t:t + 1], in_=sums[:],
                        axis=mybir.AxisListType.X, op=mybir.AluOpType.add)
                    nc.sync.dma_start(
                        out=logits_s[t * P:(t + 1) * P, :], in_=stag[:, :S])

            def sum_allreduce(half):
                h0 = half * (BT // 2)
                cc_s_in = dram.tile([P, BT // 2], F32, name=f"ccsi{half}")
                cc_s_out = dram.tile(
                    [P, BT // 2], F32, name=f"ccso{half}",
                    addr_space="Shared" if n_cores > 1 else "Local")
                nc.sync.dma_start(out=cc_s_in[:],
                                  in_=lsum[:, h0:h0 + BT // 2])
                if n_cores > 1:
                    nc.gpsimd.collective_compute(
                        "AllReduce", mybir.AluOpType.add, replica_groups=rg,
                        ins=[cc_s_in[:]], outs=[cc_s_out[:]],
                    )
                else:
                    nc.gpsimd.dma_start(out=cc_s_out[:], in_=cc_s_in[:])
                gsum = sbuf.tile([P, BT // 2], F32, name=f"gsum{half}")
                nc.sync.dma_start(out=gsum[:], in_=cc_s_out[:])
                nlogs = sbuf.tile([P, BT // 2], F32, name=f"nlogs{half}")
                nc.scalar.activation(out=nlogs[:], in_=gsum[:], func=AF.Ln)
                nc.vector.tensor_scalar_mul(nlogs[:], nlogs[:], -1.0)
                return nlogs

            def pass2_tiles(ts, nlogs, h0):
                for t in ts:
                    for k in range(NCH):
                        if CH_START[k] <= t < CH_START[k] + CHUNKS[k]:
                            break
                    lhsT = hidT[k][:, (t - CH_START[k]) * P:
                                   (t - CH_START[k] + 1) * P]
                    stag2 = stagp.tile([P, SP], BF16, tag="stag2")
                    for gi, (g0, gw) in enumerate(GROUPS):
                        ps = psum.tile([P, GRP], F32, tag="mm")
                        for s0 in range(0, gw, MMN):
                            w = min(MMN, gw - s0)
                            nc.tensor.matmul(
                                out=ps[:, s0:s0 + w], lhsT=lhsT,
                                rhs=w2_sb[:, g0 + s0:g0 + s0 + w],
                                start=True, stop=True)
                        nc.scalar.activation(
                            out=stag2[:, g0:g0 + gw], in_=ps[:, :gw],
                            func=AF.Exp, bias=nlogs[:, t - h0:t - h0 + 1])
                    nc.sync.dma_start(
                        out=soft_s[t * P:(t + 1) * P, :], in_=stag2[:, :S])

            # ---- software-pipelined emission ----
            prologue_chunk(0)
            prologue_chunk(1)
            for k in range(NCH):
                if k + 2 < NCH:
                    prologue_chunk(k + 2)
                pass1_chunk(k)
                if k == 2:  # chunks 0-2 = tiles 0-7 done
                    nlogsA = sum_allreduce(0)
            nlogsB = sum_allreduce(1)
            pass2_tiles(range(0, BT // 2), nlogsA, 0)
            pass2_tiles(range(BT // 2, BT), nlogsB, BT // 2)

    nc.compile()
    return nc


def make_in_maps(inputs: np.ndarray, W1: np.ndarray, W2: np.ndarray,
                 n_cores: int = M):
    idx = np.asarray(inputs).astype(np.int64)
    W1 = np.asarray(W1, dtype=np.float32)
    W2 = np.asarray(W2, dtype=np.float32)
    in_maps = []
    for m in range(n_cores):
        lo = m * S
        loc = idx - lo
        idxm = np.where((loc >= 0) & (loc < S), loc, S).astype(np.int32)
        idxm = np.ascontiguousarray(
            idxm.reshape(BT, P, C).transpose(1, 0, 2).reshape(P, BT * C))
        w1m = np.empty((S + 1, D), np.float16)
        w1m[:S] = W1[lo:lo + S]
        w1m[S] = 0
        w2m = np.zeros((P, SP), np.float16)
        w2m[:, :S] = W2[:, lo:lo + S]
        in_maps.append({"w1s": w1m, "w2s": w2m, "idxs": idxm})
    return in_maps


_NC_CACHE = {}


def kernel(inputs: np.ndarray, W1: np.ndarray, W2: np.ndarray):
    if "nc" not in _NC_CACHE:
        _NC_CACHE["nc"] = build_nc(M)
    nc = _NC_CACHE["nc"]
    in_maps = make_in_maps(inputs, W1, W2, M)
    res = bass_utils.run_bass_kernel_spmd(nc, in_maps, core_ids=list(range(M)))
    logits = np.empty((B, V), np.float32)
    soft = np.empty((B, V), np.float32)
    for m in range(M):
        logits[:, m * S:(m + 1) * S] = np.asarray(
            res.results[m]["logits_s"]).astype(np.float32)
        soft[:, m * S:(m + 1) * S] = np.asarray(
            res.results[m]["soft_s"]).astype(np.float32)
    return logits, soft


# revision 11
# speedup vs baseline: 1.3348x; 1.3348x over previous
"""CBOW model (embedding gather -> mean -> logits -> softmax) on 8 Trainium2
NeuronCores.

Sharding strategy (model/vocab parallel, per the hint):
  - W1 and W2 are both sharded along the vocab axis: core m owns W1 rows
    [m*12500, (m+1)*12500) (fp16, plus an appended zero row) and W2 columns
    [m*12500, (m+1)*12500) (fp16, padded to 12544 with zero columns).  Inputs
    shipped per core are ~6.5 MB instead of a replicated ~57.6 MB.
  - Gather: every core looks up ALL 2048x10 indices against its own W1 shard;
    out-of-shard indices are remapped (host-side) to the zero row, so the
    per-core context sums are partial sums.  Chunked AllReduces(add) over
    the transposed partial hidden give every core the full [128, 2048]
    hidden.  The emission is software-pipelined (prologue of chunk k+2 is
    emitted before pass 1 of chunk k) so only the first AllReduce is
    exposed, and the consumer-side reads of the AllReduce results are
    emitted just before the pass-1 chunk that needs them so no in-order
    queue head-blocks on a later collective.
  - Softmax: pass 1 computes the logit shard (written out as fp16) and
    per-row exp-sums on the f32 PSUM logits (exact Z; fused into the Exp
    activation via accum_out); AllReduces(add) of the per-row local sums
    (in two halves, so pass 2 can start while pass 1 finishes) give the
    global denominator; pass 2 recomputes the matmul tiles and applies
    exp(logit - log(sum)) via the Exp activation's per-partition bias,
    writing the softmax block as bf16.  Max-subtraction is not needed:
    |logit| < 40 always, exp() is safe in f32.  Matmuls run in fp16
    (1 cycle/row on the PE; fp32 needs 4).
"""

import numpy as np

import concourse.bass as bass
import concourse.mybir as mybir
import concourse.tile as tile
from concourse import bacc
from concourse.masks import make_identity
import concourse.bass_utils as bass_utils

# Problem shape (hardcoded; matches reference.setup_inputs()).
V = 100000      # vocab
D = 128         # embed dim
B = 2048        # batch
C = 10          # context positions
M = 8           # cores
S = V // M      # vocab shard per core = 12500
SP = 12544      # shard padded so every matmul chunk is >= 256 wide
P = 128         # partitions
BT = B // P     # batch tiles = 16
MMN = 512       # max moving free dim per matmul (one PSUM bank, f32)
GRP = 2048      # vocab columns per PSUM group (4 banks)

F32 = mybir.dt.float32
F16 = mybir.dt.float16
BF16 = mybir.dt.bfloat16
I32 = mybir.dt.int32
AF = mybir.ActivationFunctionType

# (start, width) vocab-column groups per core; width <= GRP.
GROUPS = [(g0, min(GRP, SP - g0)) for g0 in range(0, SP, GRP)]
# groups whose PSUM->SBUF logits cast runs on the scalar engine instead of
# DVE, to balance the two engines in pass 1 (ACT also does every group's exp)
ACT_COPY_GROUPS = {len(GROUPS) - 1, len(GROUPS) - 2}
# batch tiles per hidden-AllReduce chunk: a 1-tile first chunk minimizes the
# exposed pipeline head; later gathers/AllReduces overlap pass-1 compute
CHUNKS = [1, 3, 4, 4, 4]
NCH = len(CHUNKS)
CH_START = [sum(CHUNKS[:k]) for k in range(NCH)]


def build_nc(n_cores: int = M):
    nc = bacc.Bacc("TRN2", target_bir_lowering=False, debug=False,
                   num_devices=n_cores)

    w1s = nc.dram_tensor("w1s", [S + 1, D], F16, kind="ExternalInput")
    w2s = nc.dram_tensor("w2s", [P, SP], F16, kind="ExternalInput")
    idxs = nc.dram_tensor("idxs", [P, BT * C], I32, kind="ExternalInput")
    logits_s = nc.dram_tensor("logits_s", [B, S], F16, kind="ExternalOutput")
    soft_s = nc.dram_tensor("soft_s", [B, S], BF16, kind="ExternalOutput")

    rg = [list(range(n_cores))]

    with tile.TileContext(nc) as tc:
        with tc.tile_pool(name="sbuf", bufs=1) as sbuf, \
             tc.tile_pool(name="gathp", bufs=4) as gathp, \
             tc.tile_pool(name="hidp", bufs=2) as hidp, \
             tc.tile_pool(name="stagp", bufs=2) as stagp, \
             tc.tile_pool(name="psum", bufs=2, space="PSUM") as psum, \
             tc.tile_pool(name="dram", bufs=1, space="DRAM") as dram:
            idx_sb = sbuf.tile([P, BT * C], I32)
            nc.sync.dma_start(out=idx_sb[:], in_=idxs[:])

            ident = sbuf.tile([P, P], F16)
            make_identity(nc, ident[:])

            # W2 shard resident in SBUF for both passes.
            w2_sb = sbuf.tile([P, SP], F16)
            nc.sync.dma_start(out=w2_sb[:], in_=w2s[:])

            hidT = []        # per-chunk [D, ct*128] full transposed hidden
            lsum = sbuf.tile([P, BT], F32)

            def prologue_chunk(k):
                ct = CHUNKS[k]
                t0 = CH_START[k]
                hch = hidp.tile([P, ct * P], F16, tag=f"hch{ct}")
                for tt in range(ct):
                    t = t0 + tt
                    gath = gathp.tile([P, C * D], F16, tag="gath")
                    for c in range(C):
                        j = t * C + c
                        nc.gpsimd.indirect_dma_start(
                            out=gath[:, c * D:(c + 1) * D],
                            out_offset=None,
                            in_=w1s[:],
                            in_offset=bass.IndirectOffsetOnAxis(
                                ap=idx_sb[:, j:j + 1], axis=0),
                        )
                    hid = hidp.tile([P, D], F32, tag="hid")
                    nc.vector.tensor_reduce(
                        out=hid[:],
                        in_=gath[:].rearrange("p (c d) -> p d c", c=C),
                        axis=mybir.AxisListType.X,
                        op=mybir.AluOpType.add,
                    )
                    hid16 = hidp.tile([P, D], F16, tag="hid16")
                    # context mean folded in here (x 1/10)
                    nc.vector.tensor_scalar_mul(hid16[:], hid[:], 1.0 / C)
                    tp = psum.tile([P, 2 * GRP], F16, tag="mm")
                    nc.tensor.transpose(out=tp[:, :P], in_=hid16[:],
                                        identity=ident[:])
                    nc.vector.tensor_copy(hch[:, tt * P:(tt + 1) * P],
                                          tp[:, :P])
                cc_in = dram.tile([P, ct * P], F16)
                cc_out = dram.tile(
                    [P, ct * P], F16,
                    addr_space="Shared" if n_cores > 1 else "Local")
                nc.gpsimd.dma_start(out=cc_in[:], in_=hch[:])
                if n_cores > 1:
                    nc.gpsimd.collective_compute(
                        "AllReduce", mybir.AluOpType.add, replica_groups=rg,
                        ins=[cc_in[:]], outs=[cc_out[:]],
                    )
                else:
                    nc.gpsimd.dma_start(out=cc_out[:], in_=cc_in[:])
                ht = sbuf.tile([P, ct * P], F16, name=f"hidT{k}")
                hidT.append((ht, cc_out))

            def read_hidT(k):
                ht, cc_out = hidT[k]
                nc.sync.dma_start(out=ht[:], in_=cc_out[:])

            def pass1_chunk(k):
                ct = CHUNKS[k]
                t0 = CH_START[k]
                for tt in range(ct):
                    t = t0 + tt
                    lhsT = hidT[k][0][:, tt * P:(tt + 1) * P]
                    stag = stagp.tile([P, SP], F16, tag="stag1")
                    sums = hidp.tile([P, len(GROUPS)], F32, tag="sums")
                    for gi, (g0, gw) in enumerate(GROUPS):
                        ps = psum.tile([P, GRP], F32, tag="mm")
                        for s0 in range(0, gw, MMN):
                            w = min(MMN, gw - s0)
                            nc.tensor.matmul(
                                out=ps[:, s0:s0 + w], lhsT=lhsT,
                                rhs=w2_sb[:, g0 + s0:g0 + s0 + w],
                                start=True, stop=True)
                        escr = stagp.tile([P, GRP], BF16, tag="escr")
                        nc.scalar.activation(
                            out=escr[:, :gw], in_=ps[:, :gw], func=AF.Exp,
                            accum_out=sums[:, gi:gi + 1])
                        if gi in ACT_COPY_GROUPS:
                            nc.scalar.copy(stag[:, g0:g0 + gw], ps[:, :gw])
                        else:
                            nc.vector.tensor_copy(stag[:, g0:g0 + gw],
                                                  ps[:, :gw])
                    nc.vector.tensor_reduce(
                        out=lsum[:, t:t + 1], in_=sums[:],
                        axis=mybir.AxisListType.X, op=mybir.AluOpType.add)
                    nc.sync.dma_start(
                        out=logits_s[t * P:(t + 1) * P, :], in_=stag[:, :S])

            def sum_allreduce(half):
                h0 = half * (BT // 2)
                cc_s_in = dram.tile([P, BT // 2], F32, name=f"ccsi{half}")
                cc_s_out = dram.tile(
                    [P, BT // 2], F32, name=f"ccso{half}",
                    addr_space="Shared" if n_cores > 1 else "Local")
                nc.gpsimd.dma_start(out=cc_s_in[:],
                                    in_=lsum[:, h0:h0 + BT // 2])
                if n_cores > 1:
                    nc.gpsimd.collective_compute(
                        "AllReduce", mybir.AluOpType.add, replica_groups=rg,
                        ins=[cc_s_in[:]], outs=[cc_s_out[:]],
                    )
                else:
                    nc.gpsimd.dma_start(out=cc_s_out[:], in_=cc_s_in[:])
                gsum = sbuf.tile([P, BT // 2], F32, name=f"gsum{half}")
                nc.sync.dma_start(out=gsum[:], in_=cc_s_out[:])
                return gsum

            def neg_ln(gsum, half):
                nlogs = sbuf.tile([P, BT // 2], F32, name=f"nlogs{half}")
                nc.scalar.activation(out=nlogs[:], in_=gsum[:], func=AF.Ln)
                nc.vector.tensor_scalar_mul(nlogs[:], nlogs[:], -1.0)
                return nlogs

            def pass2_tiles(ts, nlogs, h0):
                for t in ts:
                    for k in range(NCH):
                        if CH_START[k] <= t < CH_START[k] + CHUNKS[k]:
                            break
                    lhsT = hidT[k][0][:, (t - CH_START[k]) * P:
                                      (t - CH_START[k] + 1) * P]
                    stag2 = stagp.tile([P, SP], BF16, tag="stag2")
                    for gi, (g0, gw) in enumerate(GROUPS):
                        ps = psum.tile([P, GRP], F32, tag="mm")
                        for s0 in range(0, gw, MMN):
                            w = min(MMN, gw - s0)
                            nc.tensor.matmul(
                                out=ps[:, s0:s0 + w], lhsT=lhsT,
                                rhs=w2_sb[:, g0 + s0:g0 + s0 + w],
                                start=True, stop=True)
                        nc.scalar.activation(
                            out=stag2[:, g0:g0 + gw], in_=ps[:, :gw],
                            func=AF.Exp, bias=nlogs[:, t - h0:t - h0 + 1])
                    nc.sync.dma_start(
                        out=soft_s[t * P:(t + 1) * P, :], in_=stag2[:, :S])

            # ---- software-pipelined emission ----
            prologue_chunk(0)
            prologue_chunk(1)
            for k in range(NCH):
                if k + 2 < NCH:
                    prologue_chunk(k + 2)
                read_hidT(k)
                pass1_chunk(k)
                if k == 2:  # chunks 0-2 = tiles 0-7 done
                    gsumA = sum_allreduce(0)
            gsumB = sum_allreduce(1)
            nlogsA = neg_ln(gsumA, 0)
            pass2_tiles(range(0, BT // 2), nlogsA, 0)
            nlogsB = neg_ln(gsumB, 1)
            pass2_tiles(range(BT // 2, BT), nlogsB, BT // 2)

    nc.compile()
    return nc


def make_in_maps(inputs: np.ndarray, W1: np.ndarray, W2: np.ndarray,
                 n_cores: int = M):
    idx = np.asarray(inputs).astype(np.int64)
    W1 = np.asarray(W1, dtype=np.float32)
    W2 = np.asarray(W2, dtype=np.float32)
    in_maps = []
    for m in range(n_cores):
        lo = m * S
        loc = idx - lo
        idxm = np.where((loc >= 0) & (loc < S), loc, S).astype(np.int32)
        idxm = np.ascontiguousarray(
            idxm.reshape(BT, P, C).transpose(1, 0, 2).reshape(P, BT * C))
        w1m = np.empty((S + 1, D), np.float16)
        w1m[:S] = W1[lo:lo + S]
        w1m[S] = 0
        w2m = np.zeros((P, SP), np.float16)
        w2m[:, :S] = W2[:, lo:lo + S]
        in_maps.append({"w1s": w1m, "w2s": w2m, "idxs": idxm})
    return in_maps


_NC_CACHE = {}


def kernel(inputs: np.ndarray, W1: np.ndarray, W2: np.ndarray):
    if "nc" not in _NC_CACHE:
        _NC_CACHE["nc"] = build_nc(M)
    nc = _NC_CACHE["nc"]
    in_maps = make_in_maps(inputs, W1, W2, M)
    res = bass_utils.run_bass_kernel_spmd(nc, in_maps, core_ids=list(range(M)))
    logits = np.empty((B, V), np.float32)
    soft = np.empty((B, V), np.float32)
    for m in range(M):
        logits[:, m * S:(m + 1) * S] = np.asarray(
            res.results[m]["logits_s"]).astype(np.float32)
        soft[:, m * S:(m + 1) * S] = np.asarray(
            res.results[m]["soft_s"]).astype(np.float32)
    return logits, soft
